# revision 8
# baseline (speedup 1.0000x reference)
"""Trainium2 Bass kernel for CausalSelfAttention with lightning (linear)
attention + LRPE, sharded over 8 NeuronCores.

Model (reference):
    qkv = x @ w_qkv.T ; split q,k,v ; per-head LRPE on q,k (dims e -> 2e)
    chunked linear attention with per-head exponential decay
    y = attn output ; out = y @ w_proj.T

Shapes: x (4, 2048, 2048), w_qkv (6144, 2048), w_proj (2048, 2048),
theta (16, 1, 128). 16 heads, head dim 128.

Sharding: 8 cores = (batch 4) x (head-group 2, 8 heads each). Each core
computes a partial output (2048, 2048) = y_part @ w_proj[:, cols].T; host
sums the two partials per batch.

Per-core pipeline:
  Phase 1 (float32r matmuls): qkT[2048, 2048] = W_qk @ x_b.T (T-layout,
           spilled to DRAM as fp16) and v_nat[2048, 1024] = x_b @ W_v.T
           (natural layout, spilled as fp16).
  Phase 2: per head: LRPE (host cos/sin tables) on DVE -> fp16, k natural
           layout via DMA-transpose (fp16), lightning attention with
           chunk=256 (exact algebraic identity vs the reference's
           chunk=128). Scores/state matmuls in fp16; decayed-state chain
           fp32; inter-term matmuls float32r. yT kept fp32r, to DRAM.
  Phase 3 (float32r): out_partial = yT.T @ w_projT
"""
import contextlib
import math

import numpy as np

import concourse.bass as bass
import concourse.tile as tile
from concourse import bacc, mybir
from concourse import bass_utils

F32 = mybir.dt.float32
F32R = mybir.dt.float32r
F16 = mybir.dt.float16

P = 128
DIM = 2048
HEADS = 16
B = 4
T = 2048
E = DIM // HEADS          # 128
HPC = HEADS // 2          # heads per core = 8
CHUNK = 256               # our chunk size (exact identity holds for any size)
NCH = T // CHUNK          # 8 chunks
KC = DIM // P             # 16 contraction chunks of 128
NT = T // 512             # 4 token tiles of 512
QK_DIMS = 2 * HPC * E     # 2048 (q then k, T-layout)
YD = HPC * E              # 1024 y dims per core

_NC_CACHE = None


def _build_nc(loop_n: int = 1, phases: str = "123"):
    """Build the (SPMD-identical) Bass program for one core.

    loop_n > 1 wraps the compute phases in a hardware loop (benchmarking
    only -- recomputes the same result loop_n times)."""
    nc = bacc.Bacc("TRN2", target_bir_lowering=False, debug=False,
                   enable_asserts=False, num_devices=8)

    xt_d = nc.dram_tensor("xt", (DIM, T), F32, kind="ExternalInput")        # x_b.T
    wt_d = nc.dram_tensor("wt", (DIM, QK_DIMS), F32, kind="ExternalInput")  # W_qk.T
    wv_d = nc.dram_tensor("wv", (DIM, YD), F32, kind="ExternalInput")       # W_v.T
    wp_d = nc.dram_tensor("wp", (YD, DIM), F32, kind="ExternalInput")       # w_proj[:, cols].T
    cos_d = nc.dram_tensor("costab", (YD, T), F32, kind="ExternalInput")
    sin_d = nc.dram_tensor("sintab", (YD, T), F32, kind="ExternalInput")
    mask_d = nc.dram_tensor("maskt", (HPC, 2, P, CHUNK), F16, kind="ExternalInput")
    qdec_d = nc.dram_tensor("qdec", (HPC, P, CHUNK), F32, kind="ExternalInput")
    kdec_d = nc.dram_tensor("kdec", (HPC, 2, P), F32, kind="ExternalInput")
    lamc_d = nc.dram_tensor("lamc", (HPC, P), F32, kind="ExternalInput")
    out_d = nc.dram_tensor("out", (T, DIM), F32, kind="ExternalOutput")

    with tile.TileContext(nc) as tc:
        with (
            tc.tile_pool(name="const", bufs=1) as constp,
            tc.tile_pool(name="dram", bufs=1, space="DRAM") as dram,
        ):
            # ---- constants (small) ----
            kdec_t = constp.tile([P, HPC, 2], F32)
            nc.sync.dma_start(kdec_t, kdec_d.ap().rearrange("h j p -> p h j"))
            lamc_t = constp.tile([P, HPC], F32)
            nc.sync.dma_start(lamc_t, lamc_d.ap().rearrange("h p -> p h"))

            qkT = dram.tile([QK_DIMS, T], F16)
            vnd = dram.tile([T, YD], F16)
            ytd = dram.tile([YD, T], F32)

            env = dict(locals())
            loop_cm = tc.For_i(0, loop_n, 1) if loop_n > 1 else contextlib.nullcontext()
            with loop_cm:
                _phases(nc, tc, env, phases)

    nc.compile()
    return nc


def _phases(nc, tc, env, which="123"):
    mult = mybir.AluOpType.mult
    add = mybir.AluOpType.add
    COPY = mybir.ActivationFunctionType.Copy
    xt_d = env["xt_d"]; wt_d = env["wt_d"]; wv_d = env["wv_d"]; wp_d = env["wp_d"]
    cos_d = env["cos_d"]; sin_d = env["sin_d"]
    mask_d = env["mask_d"]; qdec_d = env["qdec_d"]
    kdec_t = env["kdec_t"]; lamc_t = env["lamc_t"]
    qkT = env["qkT"]; vnd = env["vnd"]; ytd = env["ytd"]; out_d = env["out_d"]

    if "1" in which:
        # ====== Phase 1: qkT = W_qk @ x_b.T  and  v_nat = x_b @ W_v.T ======
        with (
            tc.tile_pool(name="p1x", bufs=1) as p1x,
            tc.tile_pool(name="p1w", bufs=2) as p1w,
            tc.tile_pool(name="p1v", bufs=1) as p1v,
            tc.tile_pool(name="p1ps", bufs=8, space="PSUM") as p1ps,
        ):
            xt = p1x.tile([P, KC, T], F32R, tag="xt")
            xt_src = xt_d.ap().rearrange("(kc p) t -> p kc t", p=P).bitcast(F32R)
            for kc in range(KC):
                nc.sync.dma_start(xt[:, kc], xt_src[:, kc])

            wt_src = wt_d.ap().rearrange("(kc p) m -> p kc m", p=P).bitcast(F32R)
            for m in range(QK_DIMS // P):           # 16
                wm = p1w.tile([P, KC, P], F32R, tag="wm")
                nc.sync.dma_start(wm, wt_src[:, :, m * P:(m + 1) * P])
                for n in range(NT):                 # 4
                    ps = p1ps.tile([P, 512], F32, tag="p1")
                    for kc in range(KC):            # 16
                        nc.tensor.matmul(ps, wm[:, kc],
                                         xt[:, kc, n * 512:(n + 1) * 512],
                                         start=(kc == 0), stop=(kc == KC - 1))
                    so = p1w.tile([P, 512], F16, tag="p1o")
                    nc.any.tensor_copy(out=so, in_=ps)
                    nc.scalar.dma_start(
                        qkT[m * P:(m + 1) * P, n * 512:(n + 1) * 512], so)

            # v in natural layout: out[tokens, vdims]
            wv_src = wv_d.ap().rearrange("(kc p) m -> p kc m", p=P).bitcast(F32R)
            for nv in range(2):                     # vdims 0:512, 512:1024
                wvt = p1v.tile([P, KC, 512], F32R, tag="wv")
                nc.sync.dma_start(wvt, wv_src[:, :, nv * 512:(nv + 1) * 512])
                for mt in range(T // P):            # 16 token tiles
                    ps = p1ps.tile([P, 512], F32, tag="p1")
                    for kc in range(KC):
                        nc.tensor.matmul(ps, xt[:, kc, mt * P:(mt + 1) * P],
                                         wvt[:, kc],
                                         start=(kc == 0), stop=(kc == KC - 1))
                    so = p1w.tile([P, 512], F16, tag="p1o")
                    nc.any.tensor_copy(out=so, in_=ps)
                    nc.scalar.dma_start(
                        vnd[mt * P:(mt + 1) * P, nv * 512:(nv + 1) * 512], so)

    if "2" in which:
        # ================= Phase 2: attention per head =================
        with (
            tc.tile_pool(name="p2cst", bufs=1) as p2cst,
            tc.tile_pool(name="p2io", bufs=2) as p2io,
            tc.tile_pool(name="p2c", bufs=3) as p2c,
            tc.tile_pool(name="p2s", bufs=2) as p2s,
            tc.tile_pool(name="p2ps", bufs=3, space="PSUM") as p2ps,
            tc.tile_pool(name="p2ps1", bufs=1, space="PSUM") as p2ps1,
        ):
            mask_t = p2cst.tile([P, HPC, 2, CHUNK], F16)
            nc.sync.dma_start(mask_t, mask_d.ap().rearrange("h j p c -> p h j c"))
            qdec_t = p2cst.tile([P, HPC, CHUNK], F32)
            nc.sync.dma_start(qdec_t, qdec_d.ap().rearrange("h p c -> p h c"))

            for h in range(HPC):
                r = h * P
                qt = p2io.tile([P, T], F16, tag="qt")
                kt = p2io.tile([P, T], F16, tag="kt")
                cost = p2io.tile([P, T], F32, tag="cost")
                sint = p2io.tile([P, T], F32, tag="sint")
                vna = p2io.tile([P, KC, E], F16, tag="vna")
                nc.sync.dma_start(qt, qkT[r:r + P])
                nc.sync.dma_start(kt, qkT[YD + r:YD + r + P])
                nc.sync.dma_start(cost, cos_d.ap()[r:r + P])
                nc.sync.dma_start(sint, sin_d.ap()[r:r + P])
                nc.sync.dma_start(
                    vna, vnd[:, r:r + P].rearrange("(tt p) d -> p tt d", p=P))

                st0 = p2s.tile([P, E], F32R, tag="st0")
                st1 = p2s.tile([P, E], F32R, tag="st1")
                ylt = p2s.tile([P, T], F32R, tag="ylt")
                lam_col = lamc_t[:, h:h + 1]

                for i in range(NCH):
                    sl = slice(i * CHUNK, (i + 1) * CHUNK)
                    # LRPE for this chunk -> fp16
                    qlc = p2c.tile([P, CHUNK], F16, tag="qlc")
                    qls = p2c.tile([P, CHUNK], F16, tag="qls")
                    klc = p2c.tile([P, CHUNK], F16, tag="klc")
                    kls = p2c.tile([P, CHUNK], F16, tag="kls")
                    nc.vector.tensor_tensor(qlc, qt[:, sl], cost[:, sl], mult)
                    nc.vector.tensor_tensor(qls, qt[:, sl], sint[:, sl], mult)
                    nc.vector.tensor_tensor(klc, kt[:, sl], cost[:, sl], mult)
                    nc.vector.tensor_tensor(kls, kt[:, sl], sint[:, sl], mult)
                    # k natural layout via DMA transpose + k_decay scale
                    ktr = p2c.tile([P, 2, 2 * E], F16, tag="ktr")
                    knat = p2c.tile([P, 2, 2 * E], F16, tag="knat")
                    for half in range(2):
                        hsl = slice(half * P, (half + 1) * P)
                        nc.scalar.dma_start_transpose(ktr[:, half, 0:E], klc[:, hsl])
                        nc.scalar.dma_start_transpose(ktr[:, half, E:2 * E], kls[:, hsl])
                        nc.scalar.activation(knat[:, half], ktr[:, half], COPY,
                                             bias=0.0,
                                             scale=kdec_t[:, h, half:half + 1])
                    # scoresT (two j-half tiles), mask multiply
                    smask = []
                    for jh in range(2):
                        jsl = slice(jh * P, (jh + 1) * P)
                        sps = p2ps.tile([P, CHUNK], F32, tag="sco")
                        nc.tensor.matmul(sps, klc[:, jsl], qlc,
                                         start=True, stop=False)
                        nc.tensor.matmul(sps, kls[:, jsl], qls,
                                         start=False, stop=True)
                        sm = p2c.tile([P, CHUNK], F16, tag=f"smask{jh}")
                        nc.vector.tensor_tensor(sm, sps, mask_t[:, h, jh], mult)
                        smask.append(sm)
                    # oT
                    ops = p2ps.tile([E, CHUNK], F32, tag="ops")
                    nc.tensor.matmul(ops, vna[:, 2 * i], smask[0],
                                     start=True, stop=False)
                    nc.tensor.matmul(ops, vna[:, 2 * i + 1], smask[1],
                                     start=False, stop=(i == 0))
                    if i > 0:
                        qsc = p2c.tile([P, CHUNK], F32R, tag="qsc")
                        qss = p2c.tile([P, CHUNK], F32R, tag="qss")
                        nc.vector.tensor_tensor(qsc, qlc, qdec_t[:, h], mult)
                        nc.vector.tensor_tensor(qss, qls, qdec_t[:, h], mult)
                        nc.tensor.matmul(ops, st0, qsc, start=False, stop=False)
                        nc.tensor.matmul(ops, st1, qss, start=False, stop=True)
                    nc.scalar.copy(ylt[:, sl], ops)
                    # state update (not needed after the last chunk)
                    if i < NCH - 1:
                        sadd0 = p2ps1.tile([E, E], F32, tag="sadd0")
                        sadd1 = p2ps1.tile([E, E], F32, tag="sadd1")
                        nc.tensor.matmul(sadd0, knat[:, 0, 0:E], vna[:, 2 * i],
                                         start=True, stop=False)
                        nc.tensor.matmul(sadd0, knat[:, 1, 0:E], vna[:, 2 * i + 1],
                                         start=False, stop=True)
                        nc.tensor.matmul(sadd1, knat[:, 0, E:2 * E], vna[:, 2 * i],
                                         start=True, stop=False)
                        nc.tensor.matmul(sadd1, knat[:, 1, E:2 * E], vna[:, 2 * i + 1],
                                         start=False, stop=True)
                        if i == 0:
                            nc.vector.tensor_copy(out=st0, in_=sadd0)
                            nc.vector.tensor_copy(out=st1, in_=sadd1)
                        else:
                            nc.vector.scalar_tensor_tensor(
                                out=st0, in0=st0.bitcast(F32), scalar=lam_col,
                                in1=sadd0, op0=mult, op1=add)
                            nc.vector.scalar_tensor_tensor(
                                out=st1, in0=st1.bitcast(F32), scalar=lam_col,
                                in1=sadd1, op0=mult, op1=add)
                nc.scalar.dma_start(ytd[r:r + P], ylt[:].bitcast(F32))

    if "3" in which:
        # ================= Phase 3: out = yT.T @ wpT =================
        with (
            tc.tile_pool(name="p3w", bufs=1) as p3w,
            tc.tile_pool(name="p3y", bufs=3) as p3y,
            tc.tile_pool(name="p3ps", bufs=8, space="PSUM") as p3ps,
        ):
            nkc3 = YD // P  # 8
            wp_src = wp_d.ap().rearrange("(kc p) n -> p kc n", p=P).bitcast(F32R)
            wpt = p3w.tile([P, nkc3, DIM], F32R, tag="wpt")
            for kc in range(nkc3):
                nc.sync.dma_start(wpt[:, kc], wp_src[:, kc])
            yt_src = ytd[:].rearrange("(kc p) t -> p kc t", p=P).bitcast(F32R)
            for m in range(T // P):                 # 16
                ym = p3y.tile([P, nkc3, P], F32R, tag="ym")
                nc.sync.dma_start(ym, yt_src[:, :, m * P:(m + 1) * P])
                for n in range(NT):                 # 4
                    ps = p3ps.tile([P, 512], F32, tag="p3")
                    for kc in range(nkc3):
                        nc.tensor.matmul(ps, ym[:, kc],
                                         wpt[:, kc, n * 512:(n + 1) * 512],
                                         start=(kc == 0), stop=(kc == nkc3 - 1))
                    so = p3y.tile([P, 512], F32, tag="p3o")
                    nc.any.tensor_copy(out=so, in_=ps)
                    nc.scalar.dma_start(
                        out_d.ap()[m * P:(m + 1) * P, n * 512:(n + 1) * 512], so)


def _get_nc():
    global _NC_CACHE
    if _NC_CACHE is None:
        _NC_CACHE = _build_nc()
    return _NC_CACHE


def _slopes(h):
    start = 2.0 ** (-(2.0 ** -(math.log2(h) - 3)))
    return np.array([start ** (i + 1) for i in range(h)], dtype=np.float64)


def _prepare_in_maps(x, w_qkv, w_proj, theta):
    slopes = _slopes(HEADS)
    t = np.arange(T, dtype=np.float64)
    idx = np.arange(CHUNK, dtype=np.float64)

    in_maps = []
    for core in range(8):
        b, g = divmod(core, 2)
        heads = np.arange(g * HPC, (g + 1) * HPC)

        xt = np.ascontiguousarray(x[b].T, dtype=np.float32)

        qk_rows = np.concatenate([
            np.arange(g * YD, (g + 1) * YD),                 # q rows
            np.arange(DIM + g * YD, DIM + (g + 1) * YD),     # k rows
        ])
        wt = np.ascontiguousarray(w_qkv[qk_rows].T, dtype=np.float32)
        v_rows = np.arange(2 * DIM + g * YD, 2 * DIM + (g + 1) * YD)
        wv = np.ascontiguousarray(w_qkv[v_rows].T, dtype=np.float32)

        wp = np.ascontiguousarray(w_proj[:, g * YD:(g + 1) * YD].T, dtype=np.float32)

        th = theta.reshape(HEADS, E)[heads].astype(np.float64)  # (8, 128)
        ang = th[:, :, None] * t[None, None, :]                 # (8, 128, T)
        costab = np.cos(ang).astype(np.float32).reshape(YD, T)
        sintab = np.sin(ang).astype(np.float32).reshape(YD, T)

        s = slopes[heads]                                       # (8,)
        diff = idx[:, None] - idx[None, :]                      # (i, j)
        maskt = np.where(
            diff[None] >= 0, np.exp(-s[:, None, None] * diff[None]), 0.0
        )                                                       # (8, i, j) = diag_decay
        maskt = np.ascontiguousarray(
            maskt.transpose(0, 2, 1).reshape(HPC, 2, P, CHUNK)).astype(np.float16)
        qdec = np.exp(-s[:, None] * (idx + 1.0)[None]).astype(np.float32)  # (8, 256)
        qdec = np.broadcast_to(qdec[:, None, :], (HPC, P, CHUNK)).copy()
        kdec = np.exp(-s[:, None] * (CHUNK - 1.0 - idx)[None]).astype(np.float32)
        kdec = np.ascontiguousarray(kdec.reshape(HPC, 2, P))
        lamc = np.exp(-s * CHUNK).astype(np.float32)            # (8,)
        lamc = np.broadcast_to(lamc[:, None], (HPC, P)).copy()

        in_maps.append({
            "xt": xt, "wt": wt, "wv": wv, "wp": wp,
            "costab": costab, "sintab": sintab,
            "maskt": maskt, "qdec": qdec, "kdec": kdec, "lamc": lamc,
        })
    return in_maps


def kernel(x, w_qkv, w_proj, theta):
    x = np.asarray(x)
    w_qkv = np.asarray(w_qkv)
    w_proj = np.asarray(w_proj)
    theta = np.asarray(theta)

    nc = _get_nc()
    in_maps = _prepare_in_maps(x, w_qkv, w_proj, theta)
    res = bass_utils.run_bass_kernel_spmd(nc, in_maps, core_ids=list(range(8)))

    out = np.empty((B, T, DIM), dtype=np.float32)
    for b in range(B):
        out[b] = res.results[2 * b]["out"] + res.results[2 * b + 1]["out"]
    return out


# revision 9
# speedup vs baseline: 1.0086x; 1.0086x over previous
"""Trainium2 Bass kernel for CausalSelfAttention with lightning (linear)
attention + LRPE, sharded over 8 NeuronCores.

Model (reference):
    qkv = x @ w_qkv.T ; split q,k,v ; per-head LRPE on q,k (dims e -> 2e)
    chunked linear attention with per-head exponential decay
    y = attn output ; out = y @ w_proj.T

Shapes: x (4, 2048, 2048), w_qkv (6144, 2048), w_proj (2048, 2048),
theta (16, 1, 128). 16 heads, head dim 128.

Sharding: 8 cores = (batch 4) x (head-group 2, 8 heads each). Each core
computes a partial output (2048, 2048) = y_part @ w_proj[:, cols].T; host
sums the two partials per batch.

Per-core pipeline:
  Phase 1 (float32r matmuls): qkT[2048, 2048] = W_qk @ x_b.T (T-layout,
           spilled to DRAM as fp16) and v_nat[2048, 1024] = x_b @ W_v.T
           (natural layout, spilled as fp16).
  Phase 2: per head: LRPE (host cos/sin tables) on DVE -> fp16, k natural
           layout via DMA-transpose (fp16), lightning attention with
           chunk=256 (exact algebraic identity vs the reference's
           chunk=128). Scores/state matmuls in fp16; decayed-state chain
           fp32; inter-term matmuls float32r. yT kept fp32r, to DRAM.
  Phase 3 (float32r): out_partial = yT.T @ w_projT
"""
import contextlib
import math

import numpy as np

import concourse.bass as bass
import concourse.tile as tile
from concourse import bacc, mybir
from concourse import bass_utils

F32 = mybir.dt.float32
F32R = mybir.dt.float32r
F16 = mybir.dt.float16

P = 128
DIM = 2048
HEADS = 16
B = 4
T = 2048
E = DIM // HEADS          # 128
HPC = HEADS // 2          # heads per core = 8
CHUNK = 256               # our chunk size (exact identity holds for any size)
NCH = T // CHUNK          # 8 chunks
KC = DIM // P             # 16 contraction chunks of 128
NT = T // 512             # 4 token tiles of 512
QK_DIMS = 2 * HPC * E     # 2048 (q then k, T-layout)
YD = HPC * E              # 1024 y dims per core

_NC_CACHE = None


def _build_nc(loop_n: int = 1, phases: str = "123"):
    """Build the (SPMD-identical) Bass program for one core.

    loop_n > 1 wraps the compute phases in a hardware loop (benchmarking
    only -- recomputes the same result loop_n times)."""
    nc = bacc.Bacc("TRN2", target_bir_lowering=False, debug=False,
                   enable_asserts=False, num_devices=8)

    xt_d = nc.dram_tensor("xt", (DIM, T), F32, kind="ExternalInput")        # x_b.T
    wt_d = nc.dram_tensor("wt", (DIM, QK_DIMS), F32, kind="ExternalInput")  # W_qk.T
    wv_d = nc.dram_tensor("wv", (DIM, YD), F32, kind="ExternalInput")       # W_v.T
    wp_d = nc.dram_tensor("wp", (YD, DIM), F32, kind="ExternalInput")       # w_proj[:, cols].T
    cos_d = nc.dram_tensor("costab", (YD, T), F32, kind="ExternalInput")
    sin_d = nc.dram_tensor("sintab", (YD, T), F32, kind="ExternalInput")
    mask_d = nc.dram_tensor("maskt", (HPC, 2, P, CHUNK), F16, kind="ExternalInput")
    qdec_d = nc.dram_tensor("qdec", (HPC, P, CHUNK), F32, kind="ExternalInput")
    kdec_d = nc.dram_tensor("kdec", (HPC, 2, P), F32, kind="ExternalInput")
    lamc_d = nc.dram_tensor("lamc", (HPC, P), F32, kind="ExternalInput")
    out_d = nc.dram_tensor("out", (T, DIM), F32, kind="ExternalOutput")

    with tile.TileContext(nc) as tc:
        with (
            tc.tile_pool(name="const", bufs=1) as constp,
            tc.tile_pool(name="dram", bufs=1, space="DRAM") as dram,
        ):
            # ---- constants (small) ----
            ident16 = constp.tile([P, P], F16)
            from concourse.masks import make_identity
            make_identity(nc, ident16)
            kdec_t = constp.tile([P, HPC, 2], F32)
            nc.sync.dma_start(kdec_t, kdec_d.ap().rearrange("h j p -> p h j"))
            lamc_t = constp.tile([P, HPC], F32)
            nc.sync.dma_start(lamc_t, lamc_d.ap().rearrange("h p -> p h"))

            qkT = dram.tile([QK_DIMS, T], F16)
            vnd = dram.tile([T, YD], F16)
            ytd = dram.tile([YD, T], F32)

            env = dict(locals())
            loop_cm = tc.For_i(0, loop_n, 1) if loop_n > 1 else contextlib.nullcontext()
            with loop_cm:
                _phases(nc, tc, env, phases)

    nc.compile()
    return nc


def _phases(nc, tc, env, which="123"):
    mult = mybir.AluOpType.mult
    add = mybir.AluOpType.add
    COPY = mybir.ActivationFunctionType.Copy
    xt_d = env["xt_d"]; wt_d = env["wt_d"]; wv_d = env["wv_d"]; wp_d = env["wp_d"]
    cos_d = env["cos_d"]; sin_d = env["sin_d"]
    mask_d = env["mask_d"]; qdec_d = env["qdec_d"]
    kdec_t = env["kdec_t"]; lamc_t = env["lamc_t"]; ident16 = env["ident16"]
    qkT = env["qkT"]; vnd = env["vnd"]; ytd = env["ytd"]; out_d = env["out_d"]

    if "1" in which:
        # ====== Phase 1: qkT = W_qk @ x_b.T  and  v_nat = x_b @ W_v.T ======
        with (
            tc.tile_pool(name="p1x", bufs=1) as p1x,
            tc.tile_pool(name="p1w", bufs=2) as p1w,
            tc.tile_pool(name="p1v", bufs=1) as p1v,
            tc.tile_pool(name="p1ps", bufs=8, space="PSUM") as p1ps,
        ):
            xt = p1x.tile([P, KC, T], F32R, tag="xt")
            xt_src = xt_d.ap().rearrange("(kc p) t -> p kc t", p=P).bitcast(F32R)
            for kc in range(KC):
                nc.sync.dma_start(xt[:, kc], xt_src[:, kc])

            wt_src = wt_d.ap().rearrange("(kc p) m -> p kc m", p=P).bitcast(F32R)
            for m in range(QK_DIMS // P):           # 16
                wm = p1w.tile([P, KC, P], F32R, tag="wm")
                nc.sync.dma_start(wm, wt_src[:, :, m * P:(m + 1) * P])
                for n in range(NT):                 # 4
                    ps = p1ps.tile([P, 512], F32, tag="p1")
                    for kc in range(KC):            # 16
                        nc.tensor.matmul(ps, wm[:, kc],
                                         xt[:, kc, n * 512:(n + 1) * 512],
                                         start=(kc == 0), stop=(kc == KC - 1))
                    so = p1w.tile([P, 512], F16, tag="p1o")
                    nc.any.tensor_copy(out=so, in_=ps)
                    nc.scalar.dma_start(
                        qkT[m * P:(m + 1) * P, n * 512:(n + 1) * 512], so)

            # v in natural layout: out[tokens, vdims]
            wv_src = wv_d.ap().rearrange("(kc p) m -> p kc m", p=P).bitcast(F32R)
            for nv in range(2):                     # vdims 0:512, 512:1024
                wvt = p1v.tile([P, KC, 512], F32R, tag="wv")
                nc.sync.dma_start(wvt, wv_src[:, :, nv * 512:(nv + 1) * 512])
                for mt in range(T // P):            # 16 token tiles
                    ps = p1ps.tile([P, 512], F32, tag="p1")
                    for kc in range(KC):
                        nc.tensor.matmul(ps, xt[:, kc, mt * P:(mt + 1) * P],
                                         wvt[:, kc],
                                         start=(kc == 0), stop=(kc == KC - 1))
                    so = p1w.tile([P, 512], F16, tag="p1o")
                    nc.any.tensor_copy(out=so, in_=ps)
                    nc.scalar.dma_start(
                        vnd[mt * P:(mt + 1) * P, nv * 512:(nv + 1) * 512], so)

    if "2" in which:
        # ================= Phase 2: attention per head =================
        with (
            tc.tile_pool(name="p2cst", bufs=1) as p2cst,
            tc.tile_pool(name="p2io", bufs=2) as p2io,
            tc.tile_pool(name="p2c", bufs=3) as p2c,
            tc.tile_pool(name="p2s", bufs=2) as p2s,
            tc.tile_pool(name="p2ps", bufs=2, space="PSUM") as p2ps,
            tc.tile_pool(name="p2ps1", bufs=1, space="PSUM") as p2ps1,
        ):
            mask_t = p2cst.tile([P, HPC, 2, CHUNK], F16)
            nc.sync.dma_start(mask_t, mask_d.ap().rearrange("h j p c -> p h j c"))
            qdec_t = p2cst.tile([P, HPC, CHUNK], F32)
            nc.sync.dma_start(qdec_t, qdec_d.ap().rearrange("h p c -> p h c"))

            for h in range(HPC):
                r = h * P
                qt = p2io.tile([P, T], F16, tag="qt")
                kt = p2io.tile([P, T], F16, tag="kt")
                cost = p2io.tile([P, T], F32, tag="cost")
                sint = p2io.tile([P, T], F32, tag="sint")
                vna = p2io.tile([P, KC, E], F16, tag="vna")
                nc.sync.dma_start(qt, qkT[r:r + P])
                nc.sync.dma_start(kt, qkT[YD + r:YD + r + P])
                nc.sync.dma_start(cost, cos_d.ap()[r:r + P])
                nc.sync.dma_start(sint, sin_d.ap()[r:r + P])
                nc.sync.dma_start(
                    vna, vnd[:, r:r + P].rearrange("(tt p) d -> p tt d", p=P))

                st0 = p2s.tile([P, E], F32R, tag="st0")
                st1 = p2s.tile([P, E], F32R, tag="st1")
                ylt = p2s.tile([P, T], F32R, tag="ylt")
                lam_col = lamc_t[:, h:h + 1]

                for i in range(NCH):
                    sl = slice(i * CHUNK, (i + 1) * CHUNK)
                    # LRPE for this chunk -> fp16
                    qlc = p2c.tile([P, CHUNK], F16, tag="qlc")
                    qls = p2c.tile([P, CHUNK], F16, tag="qls")
                    klc = p2c.tile([P, CHUNK], F16, tag="klc")
                    kls = p2c.tile([P, CHUNK], F16, tag="kls")
                    nc.vector.tensor_tensor(qlc, qt[:, sl], cost[:, sl], mult)
                    nc.vector.tensor_tensor(qls, qt[:, sl], sint[:, sl], mult)
                    nc.vector.tensor_tensor(klc, kt[:, sl], cost[:, sl], mult)
                    nc.vector.tensor_tensor(kls, kt[:, sl], sint[:, sl], mult)
                    # k natural layout via PE transpose (fp16) + k_decay scale
                    knat = p2c.tile([P, 2, 2 * E], F16, tag="knat")
                    for half in range(2):
                        hsl = slice(half * P, (half + 1) * P)
                        kd = kdec_t[:, h, half:half + 1]
                        pk0 = p2ps.tile([P, P], F16, tag="ptr")
                        nc.tensor.transpose(pk0, klc[:, hsl], ident16)
                        nc.scalar.activation(knat[:, half, 0:E], pk0, COPY,
                                             bias=0.0, scale=kd)
                        pk1 = p2ps.tile([P, P], F16, tag="ptr")
                        nc.tensor.transpose(pk1, kls[:, hsl], ident16)
                        nc.scalar.activation(knat[:, half, E:2 * E], pk1, COPY,
                                             bias=0.0, scale=kd)
                    # scoresT (two j-half tiles), mask multiply
                    smask = []
                    for jh in range(2):
                        jsl = slice(jh * P, (jh + 1) * P)
                        sps = p2ps.tile([P, CHUNK], F32, tag="sco")
                        nc.tensor.matmul(sps, klc[:, jsl], qlc,
                                         start=True, stop=False)
                        nc.tensor.matmul(sps, kls[:, jsl], qls,
                                         start=False, stop=True)
                        sm = p2c.tile([P, CHUNK], F16, tag=f"smask{jh}")
                        nc.vector.tensor_tensor(sm, sps, mask_t[:, h, jh], mult)
                        smask.append(sm)
                    # oT
                    ops = p2ps.tile([E, CHUNK], F32, tag="ops")
                    nc.tensor.matmul(ops, vna[:, 2 * i], smask[0],
                                     start=True, stop=False)
                    nc.tensor.matmul(ops, vna[:, 2 * i + 1], smask[1],
                                     start=False, stop=(i == 0))
                    if i > 0:
                        qsc = p2c.tile([P, CHUNK], F32R, tag="qsc")
                        qss = p2c.tile([P, CHUNK], F32R, tag="qss")
                        nc.vector.tensor_tensor(qsc, qlc, qdec_t[:, h], mult)
                        nc.vector.tensor_tensor(qss, qls, qdec_t[:, h], mult)
                        nc.tensor.matmul(ops, st0, qsc, start=False, stop=False)
                        nc.tensor.matmul(ops, st1, qss, start=False, stop=True)
                    nc.scalar.copy(ylt[:, sl], ops)
                    # state update (not needed after the last chunk)
                    if i < NCH - 1:
                        sadd0 = p2ps1.tile([E, E], F32, tag="sadd0")
                        sadd1 = p2ps1.tile([E, E], F32, tag="sadd1")
                        nc.tensor.matmul(sadd0, knat[:, 0, 0:E], vna[:, 2 * i],
                                         start=True, stop=False)
                        nc.tensor.matmul(sadd0, knat[:, 1, 0:E], vna[:, 2 * i + 1],
                                         start=False, stop=True)
                        nc.tensor.matmul(sadd1, knat[:, 0, E:2 * E], vna[:, 2 * i],
                                         start=True, stop=False)
                        nc.tensor.matmul(sadd1, knat[:, 1, E:2 * E], vna[:, 2 * i + 1],
                                         start=False, stop=True)
                        if i == 0:
                            nc.vector.tensor_copy(out=st0, in_=sadd0)
                            nc.vector.tensor_copy(out=st1, in_=sadd1)
                        else:
                            nc.vector.scalar_tensor_tensor(
                                out=st0, in0=st0.bitcast(F32), scalar=lam_col,
                                in1=sadd0, op0=mult, op1=add)
                            nc.vector.scalar_tensor_tensor(
                                out=st1, in0=st1.bitcast(F32), scalar=lam_col,
                                in1=sadd1, op0=mult, op1=add)
                nc.scalar.dma_start(ytd[r:r + P], ylt[:].bitcast(F32))

    if "3" in which:
        # ================= Phase 3: out = yT.T @ wpT =================
        with (
            tc.tile_pool(name="p3w", bufs=1) as p3w,
            tc.tile_pool(name="p3y", bufs=3) as p3y,
            tc.tile_pool(name="p3ps", bufs=8, space="PSUM") as p3ps,
        ):
            nkc3 = YD // P  # 8
            wp_src = wp_d.ap().rearrange("(kc p) n -> p kc n", p=P).bitcast(F32R)
            wpt = p3w.tile([P, nkc3, DIM], F32R, tag="wpt")
            for kc in range(nkc3):
                nc.sync.dma_start(wpt[:, kc], wp_src[:, kc])
            yt_src = ytd[:].rearrange("(kc p) t -> p kc t", p=P).bitcast(F32R)
            for m in range(T // P):                 # 16
                ym = p3y.tile([P, nkc3, P], F32R, tag="ym")
                nc.sync.dma_start(ym, yt_src[:, :, m * P:(m + 1) * P])
                for n in range(NT):                 # 4
                    ps = p3ps.tile([P, 512], F32, tag="p3")
                    for kc in range(nkc3):
                        nc.tensor.matmul(ps, ym[:, kc],
                                         wpt[:, kc, n * 512:(n + 1) * 512],
                                         start=(kc == 0), stop=(kc == nkc3 - 1))
                    so = p3y.tile([P, 512], F32, tag="p3o")
                    nc.any.tensor_copy(out=so, in_=ps)
                    nc.scalar.dma_start(
                        out_d.ap()[m * P:(m + 1) * P, n * 512:(n + 1) * 512], so)


def _get_nc():
    global _NC_CACHE
    if _NC_CACHE is None:
        _NC_CACHE = _build_nc()
    return _NC_CACHE


def _slopes(h):
    start = 2.0 ** (-(2.0 ** -(math.log2(h) - 3)))
    return np.array([start ** (i + 1) for i in range(h)], dtype=np.float64)


def _prepare_in_maps(x, w_qkv, w_proj, theta):
    slopes = _slopes(HEADS)
    t = np.arange(T, dtype=np.float64)
    idx = np.arange(CHUNK, dtype=np.float64)

    in_maps = []
    for core in range(8):
        b, g = divmod(core, 2)
        heads = np.arange(g * HPC, (g + 1) * HPC)

        xt = np.ascontiguousarray(x[b].T, dtype=np.float32)

        qk_rows = np.concatenate([
            np.arange(g * YD, (g + 1) * YD),                 # q rows
            np.arange(DIM + g * YD, DIM + (g + 1) * YD),     # k rows
        ])
        wt = np.ascontiguousarray(w_qkv[qk_rows].T, dtype=np.float32)
        v_rows = np.arange(2 * DIM + g * YD, 2 * DIM + (g + 1) * YD)
        wv = np.ascontiguousarray(w_qkv[v_rows].T, dtype=np.float32)

        wp = np.ascontiguousarray(w_proj[:, g * YD:(g + 1) * YD].T, dtype=np.float32)

        th = theta.reshape(HEADS, E)[heads].astype(np.float64)  # (8, 128)
        ang = th[:, :, None] * t[None, None, :]                 # (8, 128, T)
        costab = np.cos(ang).astype(np.float32).reshape(YD, T)
        sintab = np.sin(ang).astype(np.float32).reshape(YD, T)

        s = slopes[heads]                                       # (8,)
        diff = idx[:, None] - idx[None, :]                      # (i, j)
        maskt = np.where(
            diff[None] >= 0, np.exp(-s[:, None, None] * diff[None]), 0.0
        )                                                       # (8, i, j) = diag_decay
        maskt = np.ascontiguousarray(
            maskt.transpose(0, 2, 1).reshape(HPC, 2, P, CHUNK)).astype(np.float16)
        qdec = np.exp(-s[:, None] * (idx + 1.0)[None]).astype(np.float32)  # (8, 256)
        qdec = np.broadcast_to(qdec[:, None, :], (HPC, P, CHUNK)).copy()
        kdec = np.exp(-s[:, None] * (CHUNK - 1.0 - idx)[None]).astype(np.float32)
        kdec = np.ascontiguousarray(kdec.reshape(HPC, 2, P))
        lamc = np.exp(-s * CHUNK).astype(np.float32)            # (8,)
        lamc = np.broadcast_to(lamc[:, None], (HPC, P)).copy()

        in_maps.append({
            "xt": xt, "wt": wt, "wv": wv, "wp": wp,
            "costab": costab, "sintab": sintab,
            "maskt": maskt, "qdec": qdec, "kdec": kdec, "lamc": lamc,
        })
    return in_maps


def kernel(x, w_qkv, w_proj, theta):
    x = np.asarray(x)
    w_qkv = np.asarray(w_qkv)
    w_proj = np.asarray(w_proj)
    theta = np.asarray(theta)

    nc = _get_nc()
    in_maps = _prepare_in_maps(x, w_qkv, w_proj, theta)
    res = bass_utils.run_bass_kernel_spmd(nc, in_maps, core_ids=list(range(8)))

    out = np.empty((B, T, DIM), dtype=np.float32)
    for b in range(B):
        out[b] = res.results[2 * b]["out"] + res.results[2 * b + 1]["out"]
    return out


# revision 10
# speedup vs baseline: 1.0611x; 1.0521x over previous
"""Trainium2 Bass kernel for CausalSelfAttention with lightning (linear)
attention + LRPE, sharded over 8 NeuronCores.

Model (reference):
    qkv = x @ w_qkv.T ; split q,k,v ; per-head LRPE on q,k (dims e -> 2e)
    chunked linear attention with per-head exponential decay
    y = attn output ; out = y @ w_proj.T

Shapes: x (4, 2048, 2048), w_qkv (6144, 2048), w_proj (2048, 2048),
theta (16, 1, 128). 16 heads, head dim 128.

Sharding: 8 cores = (batch 4) x (head-group 2, 8 heads each). Each core
computes a partial output (2048, 2048) = y_part @ w_proj[:, cols].T; host
sums the two partials per batch.

Per-core pipeline:
  Phase 1 (float32r matmuls): qkT[2048, 2048] = W_qk @ x_b.T (T-layout,
           spilled to DRAM as fp16) and v_nat[2048, 1024] = x_b @ W_v.T
           (natural layout, spilled as fp16).
  Phase 2: per head: LRPE (host cos/sin tables) on DVE -> fp16, k natural
           layout via DMA-transpose (fp16), lightning attention with
           chunk=256 (exact algebraic identity vs the reference's
           chunk=128). Scores/state matmuls in fp16; decayed-state chain
           fp32; inter-term matmuls float32r. yT kept fp32r, to DRAM.
  Phase 3 (float32r): out_partial = yT.T @ w_projT
"""
import contextlib
import math

import numpy as np

import concourse.bass as bass
import concourse.tile as tile
from concourse import bacc, mybir
from concourse import bass_utils

F32 = mybir.dt.float32
F32R = mybir.dt.float32r
F16 = mybir.dt.float16

P = 128
DIM = 2048
HEADS = 16
B = 4
T = 2048
E = DIM // HEADS          # 128
HPC = HEADS // 2          # heads per core = 8
CHUNK = 256               # our chunk size (exact identity holds for any size)
NCH = T // CHUNK          # 8 chunks
KC = DIM // P             # 16 contraction chunks of 128
NT = T // 512             # 4 token tiles of 512
QK_DIMS = 2 * HPC * E     # 2048 (q then k, T-layout)
YD = HPC * E              # 1024 y dims per core

_NC_CACHE = None


def _build_nc(loop_n: int = 1, phases: str = "123"):
    """Build the (SPMD-identical) Bass program for one core.

    loop_n > 1 wraps the compute phases in a hardware loop (benchmarking
    only -- recomputes the same result loop_n times)."""
    nc = bacc.Bacc("TRN2", target_bir_lowering=False, debug=False,
                   enable_asserts=False, num_devices=8)

    xt_d = nc.dram_tensor("xt", (DIM, T), F16, kind="ExternalInput")        # x_b.T
    wt_d = nc.dram_tensor("wt", (DIM, QK_DIMS), F16, kind="ExternalInput")  # W_qk.T
    wv_d = nc.dram_tensor("wv", (DIM, YD), F16, kind="ExternalInput")       # W_v.T
    wp_d = nc.dram_tensor("wp", (YD, DIM), F32, kind="ExternalInput")       # w_proj[:, cols].T
    cos_d = nc.dram_tensor("costab", (YD, T), F32, kind="ExternalInput")
    sin_d = nc.dram_tensor("sintab", (YD, T), F32, kind="ExternalInput")
    mask_d = nc.dram_tensor("maskt", (HPC, 2, P, CHUNK), F16, kind="ExternalInput")
    qdec_d = nc.dram_tensor("qdec", (HPC, P, CHUNK), F32, kind="ExternalInput")
    kdec_d = nc.dram_tensor("kdec", (HPC, 2, P), F32, kind="ExternalInput")
    lamc_d = nc.dram_tensor("lamc", (HPC, P), F32, kind="ExternalInput")
    out_d = nc.dram_tensor("out", (T, DIM), F32, kind="ExternalOutput")

    with tile.TileContext(nc) as tc:
        with (
            tc.tile_pool(name="const", bufs=1) as constp,
            tc.tile_pool(name="dram", bufs=1, space="DRAM") as dram,
        ):
            # ---- constants (small) ----
            ident16 = constp.tile([P, P], F16)
            from concourse.masks import make_identity
            make_identity(nc, ident16)
            kdec_t = constp.tile([P, HPC, 2], F32)
            nc.sync.dma_start(kdec_t, kdec_d.ap().rearrange("h j p -> p h j"))
            lamc_t = constp.tile([P, HPC], F32)
            nc.sync.dma_start(lamc_t, lamc_d.ap().rearrange("h p -> p h"))

            qkT = dram.tile([QK_DIMS, T], F16)
            vnd = dram.tile([T, YD], F16)
            ytd = dram.tile([YD, T], F32)

            env = dict(locals())
            loop_cm = tc.For_i(0, loop_n, 1) if loop_n > 1 else contextlib.nullcontext()
            with loop_cm:
                _phases(nc, tc, env, phases)

    nc.compile()
    return nc


def _phases(nc, tc, env, which="123"):
    mult = mybir.AluOpType.mult
    add = mybir.AluOpType.add
    COPY = mybir.ActivationFunctionType.Copy
    xt_d = env["xt_d"]; wt_d = env["wt_d"]; wv_d = env["wv_d"]; wp_d = env["wp_d"]
    cos_d = env["cos_d"]; sin_d = env["sin_d"]
    mask_d = env["mask_d"]; qdec_d = env["qdec_d"]
    kdec_t = env["kdec_t"]; lamc_t = env["lamc_t"]; ident16 = env["ident16"]
    qkT = env["qkT"]; vnd = env["vnd"]; ytd = env["ytd"]; out_d = env["out_d"]

    if "1" in which:
        # ====== Phase 1: qkT = W_qk @ x_b.T  and  v_nat = x_b @ W_v.T ======
        with (
            tc.tile_pool(name="p1x", bufs=1) as p1x,
            tc.tile_pool(name="p1w", bufs=2) as p1w,
            tc.tile_pool(name="p1v", bufs=1) as p1v,
            tc.tile_pool(name="p1ps", bufs=8, space="PSUM") as p1ps,
        ):
            xt = p1x.tile([P, KC, T], F16, tag="xt")
            xt_src = xt_d.ap().rearrange("(kc p) t -> p kc t", p=P)
            for kc in range(KC):
                nc.sync.dma_start(xt[:, kc], xt_src[:, kc])

            wt_src = wt_d.ap().rearrange("(kc p) m -> p kc m", p=P)
            for m in range(QK_DIMS // P):           # 16
                wm = p1w.tile([P, KC, P], F16, tag="wm")
                nc.sync.dma_start(wm, wt_src[:, :, m * P:(m + 1) * P])
                for n in range(NT):                 # 4
                    ps = p1ps.tile([P, 512], F32, tag="p1")
                    for kc in range(KC):            # 16
                        nc.tensor.matmul(ps, wm[:, kc],
                                         xt[:, kc, n * 512:(n + 1) * 512],
                                         start=(kc == 0), stop=(kc == KC - 1))
                    so = p1w.tile([P, 512], F16, tag="p1o")
                    nc.any.tensor_copy(out=so, in_=ps)
                    nc.scalar.dma_start(
                        qkT[m * P:(m + 1) * P, n * 512:(n + 1) * 512], so)

            # v in natural layout: out[tokens, vdims]
            wv_src = wv_d.ap().rearrange("(kc p) m -> p kc m", p=P)
            for nv in range(2):                     # vdims 0:512, 512:1024
                wvt = p1v.tile([P, KC, 512], F16, tag="wv")
                nc.sync.dma_start(wvt, wv_src[:, :, nv * 512:(nv + 1) * 512])
                for mt in range(T // P):            # 16 token tiles
                    ps = p1ps.tile([P, 512], F32, tag="p1")
                    for kc in range(KC):
                        nc.tensor.matmul(ps, xt[:, kc, mt * P:(mt + 1) * P],
                                         wvt[:, kc],
                                         start=(kc == 0), stop=(kc == KC - 1))
                    so = p1w.tile([P, 512], F16, tag="p1o")
                    nc.any.tensor_copy(out=so, in_=ps)
                    nc.scalar.dma_start(
                        vnd[mt * P:(mt + 1) * P, nv * 512:(nv + 1) * 512], so)

    if "2" in which:
        # ================= Phase 2: attention per head =================
        with (
            tc.tile_pool(name="p2cst", bufs=1) as p2cst,
            tc.tile_pool(name="p2io", bufs=2) as p2io,
            tc.tile_pool(name="p2c", bufs=3) as p2c,
            tc.tile_pool(name="p2s", bufs=2) as p2s,
            tc.tile_pool(name="p2ps", bufs=2, space="PSUM") as p2ps,
            tc.tile_pool(name="p2ps1", bufs=1, space="PSUM") as p2ps1,
        ):
            mask_t = p2cst.tile([P, HPC, 2, CHUNK], F16)
            nc.sync.dma_start(mask_t, mask_d.ap().rearrange("h j p c -> p h j c"))
            qdec_t = p2cst.tile([P, HPC, CHUNK], F32)
            nc.sync.dma_start(qdec_t, qdec_d.ap().rearrange("h p c -> p h c"))

            for h in range(HPC):
                r = h * P
                qt = p2io.tile([P, T], F16, tag="qt")
                kt = p2io.tile([P, T], F16, tag="kt")
                cost = p2io.tile([P, T], F32, tag="cost")
                sint = p2io.tile([P, T], F32, tag="sint")
                vna = p2io.tile([P, KC, E], F16, tag="vna")
                nc.sync.dma_start(qt, qkT[r:r + P])
                nc.sync.dma_start(kt, qkT[YD + r:YD + r + P])
                nc.sync.dma_start(cost, cos_d.ap()[r:r + P])
                nc.sync.dma_start(sint, sin_d.ap()[r:r + P])
                nc.sync.dma_start(
                    vna, vnd[:, r:r + P].rearrange("(tt p) d -> p tt d", p=P))

                st0 = p2s.tile([P, E], F32R, tag="st0")
                st1 = p2s.tile([P, E], F32R, tag="st1")
                ylt = p2s.tile([P, T], F32R, tag="ylt")
                lam_col = lamc_t[:, h:h + 1]

                for i in range(NCH):
                    sl = slice(i * CHUNK, (i + 1) * CHUNK)
                    # LRPE for this chunk -> fp16
                    qlc = p2c.tile([P, CHUNK], F16, tag="qlc")
                    qls = p2c.tile([P, CHUNK], F16, tag="qls")
                    klc = p2c.tile([P, CHUNK], F16, tag="klc")
                    kls = p2c.tile([P, CHUNK], F16, tag="kls")
                    nc.vector.tensor_tensor(qlc, qt[:, sl], cost[:, sl], mult)
                    nc.vector.tensor_tensor(qls, qt[:, sl], sint[:, sl], mult)
                    nc.vector.tensor_tensor(klc, kt[:, sl], cost[:, sl], mult)
                    nc.vector.tensor_tensor(kls, kt[:, sl], sint[:, sl], mult)
                    # k natural layout via PE transpose (fp16) + k_decay scale
                    knat = p2c.tile([P, 2, 2 * E], F16, tag="knat")
                    for half in range(2):
                        hsl = slice(half * P, (half + 1) * P)
                        kd = kdec_t[:, h, half:half + 1]
                        pk0 = p2ps.tile([P, P], F16, tag="ptr")
                        nc.tensor.transpose(pk0, klc[:, hsl], ident16)
                        nc.scalar.activation(knat[:, half, 0:E], pk0, COPY,
                                             bias=0.0, scale=kd)
                        pk1 = p2ps.tile([P, P], F16, tag="ptr")
                        nc.tensor.transpose(pk1, kls[:, hsl], ident16)
                        nc.scalar.activation(knat[:, half, E:2 * E], pk1, COPY,
                                             bias=0.0, scale=kd)
                    # scoresT (two j-half tiles), mask multiply
                    smask = []
                    for jh in range(2):
                        jsl = slice(jh * P, (jh + 1) * P)
                        sps = p2ps.tile([P, CHUNK], F32, tag="sco")
                        nc.tensor.matmul(sps, klc[:, jsl], qlc,
                                         start=True, stop=False)
                        nc.tensor.matmul(sps, kls[:, jsl], qls,
                                         start=False, stop=True)
                        sm = p2c.tile([P, CHUNK], F16, tag=f"smask{jh}")
                        nc.vector.tensor_tensor(sm, sps, mask_t[:, h, jh], mult)
                        smask.append(sm)
                    # oT
                    ops = p2ps.tile([E, CHUNK], F32, tag="ops")
                    nc.tensor.matmul(ops, vna[:, 2 * i], smask[0],
                                     start=True, stop=False)
                    nc.tensor.matmul(ops, vna[:, 2 * i + 1], smask[1],
                                     start=False, stop=(i == 0))
                    if i > 0:
                        qsc = p2c.tile([P, CHUNK], F32R, tag="qsc")
                        qss = p2c.tile([P, CHUNK], F32R, tag="qss")
                        nc.vector.tensor_tensor(qsc, qlc, qdec_t[:, h], mult)
                        nc.vector.tensor_tensor(qss, qls, qdec_t[:, h], mult)
                        nc.tensor.matmul(ops, st0, qsc, start=False, stop=False)
                        nc.tensor.matmul(ops, st1, qss, start=False, stop=True)
                    nc.scalar.copy(ylt[:, sl], ops)
                    # state update (not needed after the last chunk)
                    if i < NCH - 1:
                        sadd0 = p2ps1.tile([E, E], F32, tag="sadd0")
                        sadd1 = p2ps1.tile([E, E], F32, tag="sadd1")
                        nc.tensor.matmul(sadd0, knat[:, 0, 0:E], vna[:, 2 * i],
                                         start=True, stop=False)
                        nc.tensor.matmul(sadd0, knat[:, 1, 0:E], vna[:, 2 * i + 1],
                                         start=False, stop=True)
                        nc.tensor.matmul(sadd1, knat[:, 0, E:2 * E], vna[:, 2 * i],
                                         start=True, stop=False)
                        nc.tensor.matmul(sadd1, knat[:, 1, E:2 * E], vna[:, 2 * i + 1],
                                         start=False, stop=True)
                        if i == 0:
                            nc.vector.tensor_copy(out=st0, in_=sadd0)
                            nc.vector.tensor_copy(out=st1, in_=sadd1)
                        else:
                            nc.vector.scalar_tensor_tensor(
                                out=st0, in0=st0.bitcast(F32), scalar=lam_col,
                                in1=sadd0, op0=mult, op1=add)
                            nc.vector.scalar_tensor_tensor(
                                out=st1, in0=st1.bitcast(F32), scalar=lam_col,
                                in1=sadd1, op0=mult, op1=add)
                nc.scalar.dma_start(ytd[r:r + P], ylt[:].bitcast(F32))

    if "3" in which:
        # ================= Phase 3: out = yT.T @ wpT =================
        with (
            tc.tile_pool(name="p3w", bufs=1) as p3w,
            tc.tile_pool(name="p3y", bufs=3) as p3y,
            tc.tile_pool(name="p3ps", bufs=8, space="PSUM") as p3ps,
        ):
            nkc3 = YD // P  # 8
            wp_src = wp_d.ap().rearrange("(kc p) n -> p kc n", p=P).bitcast(F32R)
            wpt = p3w.tile([P, nkc3, DIM], F32R, tag="wpt")
            for kc in range(nkc3):
                nc.sync.dma_start(wpt[:, kc], wp_src[:, kc])
            yt_src = ytd[:].rearrange("(kc p) t -> p kc t", p=P).bitcast(F32R)
            for m in range(T // P):                 # 16
                ym = p3y.tile([P, nkc3, P], F32R, tag="ym")
                nc.sync.dma_start(ym, yt_src[:, :, m * P:(m + 1) * P])
                for n in range(NT):                 # 4
                    ps = p3ps.tile([P, 512], F32, tag="p3")
                    for kc in range(nkc3):
                        nc.tensor.matmul(ps, ym[:, kc],
                                         wpt[:, kc, n * 512:(n + 1) * 512],
                                         start=(kc == 0), stop=(kc == nkc3 - 1))
                    so = p3y.tile([P, 512], F32, tag="p3o")
                    nc.any.tensor_copy(out=so, in_=ps)
                    nc.scalar.dma_start(
                        out_d.ap()[m * P:(m + 1) * P, n * 512:(n + 1) * 512], so)


def _get_nc():
    global _NC_CACHE
    if _NC_CACHE is None:
        _NC_CACHE = _build_nc()
    return _NC_CACHE


def _slopes(h):
    start = 2.0 ** (-(2.0 ** -(math.log2(h) - 3)))
    return np.array([start ** (i + 1) for i in range(h)], dtype=np.float64)


def _prepare_in_maps(x, w_qkv, w_proj, theta):
    slopes = _slopes(HEADS)
    t = np.arange(T, dtype=np.float64)
    idx = np.arange(CHUNK, dtype=np.float64)

    in_maps = []
    for core in range(8):
        b, g = divmod(core, 2)
        heads = np.arange(g * HPC, (g + 1) * HPC)

        xt = np.ascontiguousarray(x[b].T).astype(np.float16)

        qk_rows = np.concatenate([
            np.arange(g * YD, (g + 1) * YD),                 # q rows
            np.arange(DIM + g * YD, DIM + (g + 1) * YD),     # k rows
        ])
        wt = np.ascontiguousarray(w_qkv[qk_rows].T).astype(np.float16)
        v_rows = np.arange(2 * DIM + g * YD, 2 * DIM + (g + 1) * YD)
        wv = np.ascontiguousarray(w_qkv[v_rows].T).astype(np.float16)

        wp = np.ascontiguousarray(w_proj[:, g * YD:(g + 1) * YD].T, dtype=np.float32)

        th = theta.reshape(HEADS, E)[heads].astype(np.float64)  # (8, 128)
        ang = th[:, :, None] * t[None, None, :]                 # (8, 128, T)
        costab = np.cos(ang).astype(np.float32).reshape(YD, T)
        sintab = np.sin(ang).astype(np.float32).reshape(YD, T)

        s = slopes[heads]                                       # (8,)
        diff = idx[:, None] - idx[None, :]                      # (i, j)
        maskt = np.where(
            diff[None] >= 0, np.exp(-s[:, None, None] * diff[None]), 0.0
        )                                                       # (8, i, j) = diag_decay
        maskt = np.ascontiguousarray(
            maskt.transpose(0, 2, 1).reshape(HPC, 2, P, CHUNK)).astype(np.float16)
        qdec = np.exp(-s[:, None] * (idx + 1.0)[None]).astype(np.float32)  # (8, 256)
        qdec = np.broadcast_to(qdec[:, None, :], (HPC, P, CHUNK)).copy()
        kdec = np.exp(-s[:, None] * (CHUNK - 1.0 - idx)[None]).astype(np.float32)
        kdec = np.ascontiguousarray(kdec.reshape(HPC, 2, P))
        lamc = np.exp(-s * CHUNK).astype(np.float32)            # (8,)
        lamc = np.broadcast_to(lamc[:, None], (HPC, P)).copy()

        in_maps.append({
            "xt": xt, "wt": wt, "wv": wv, "wp": wp,
            "costab": costab, "sintab": sintab,
            "maskt": maskt, "qdec": qdec, "kdec": kdec, "lamc": lamc,
        })
    return in_maps


def kernel(x, w_qkv, w_proj, theta):
    x = np.asarray(x)
    w_qkv = np.asarray(w_qkv)
    w_proj = np.asarray(w_proj)
    theta = np.asarray(theta)

    nc = _get_nc()
    in_maps = _prepare_in_maps(x, w_qkv, w_proj, theta)
    res = bass_utils.run_bass_kernel_spmd(nc, in_maps, core_ids=list(range(8)))

    out = np.empty((B, T, DIM), dtype=np.float32)
    for b in range(B):
        out[b] = res.results[2 * b]["out"] + res.results[2 * b + 1]["out"]
    return out


# revision 13
# speedup vs baseline: 1.4442x; 1.3610x over previous
"""Trainium2 Bass kernel for CausalSelfAttention with lightning (linear)
attention + LRPE, sharded over 8 NeuronCores.

Model (reference):
    qkv = x @ w_qkv.T ; split q,k,v ; per-head LRPE on q,k (dims e -> 2e)
    chunked linear attention with per-head exponential decay
    y = attn output ; out = y @ w_proj.T

Shapes: x (4, 2048, 2048), w_qkv (6144, 2048), w_proj (2048, 2048),
theta (16, 1, 128). 16 heads, head dim 128.

Sharding: 8 cores = (batch 4) x (head-group 2, 8 heads each). Each core
computes a partial output (2048, 2048) = y_part @ w_proj[:, cols].T; host
sums the two partials per batch.

Per-core pipeline:
  Phase 1 (float32r matmuls): qkT[2048, 2048] = W_qk @ x_b.T (T-layout,
           spilled to DRAM as fp16) and v_nat[2048, 1024] = x_b @ W_v.T
           (natural layout, spilled as fp16).
  Phase 2: per head: LRPE (host cos/sin tables) on DVE -> fp16, k natural
           layout via DMA-transpose (fp16), lightning attention with
           chunk=256 (exact algebraic identity vs the reference's
           chunk=128). Scores/state matmuls in fp16; decayed-state chain
           fp32; inter-term matmuls float32r. yT kept fp32r, to DRAM.
  Phase 3 (float32r): out_partial = yT.T @ w_projT
"""
import contextlib
import math

import numpy as np

import concourse.bass as bass
import concourse.tile as tile
from concourse import bacc, mybir
from concourse import bass_utils

F32 = mybir.dt.float32
F32R = mybir.dt.float32r
F16 = mybir.dt.float16

P = 128
DIM = 2048
HEADS = 16
B = 4
T = 2048
E = DIM // HEADS          # 128
HPC = HEADS // 2          # heads per core = 8
CHUNK = 256               # our chunk size (exact identity holds for any size)
NCH = T // CHUNK          # 8 chunks
KC = DIM // P             # 16 contraction chunks of 128
NT = T // 512             # 4 token tiles of 512
QK_DIMS = 2 * HPC * E     # 2048 (q then k, T-layout)
YD = HPC * E              # 1024 y dims per core

_NC_CACHE = None


def _build_nc(loop_n: int = 1, phases: str = "123"):
    """Build the (SPMD-identical) Bass program for one core.

    loop_n > 1 wraps the compute phases in a hardware loop (benchmarking
    only -- recomputes the same result loop_n times)."""
    nc = bacc.Bacc("TRN2", target_bir_lowering=False, debug=False,
                   enable_asserts=False, num_devices=8)

    xt_d = nc.dram_tensor("xt", (DIM, T), F16, kind="ExternalInput")        # x_b.T
    wt_d = nc.dram_tensor("wt", (DIM, QK_DIMS), F16, kind="ExternalInput")  # W_qk.T
    wv_d = nc.dram_tensor("wv", (DIM, YD), F16, kind="ExternalInput")       # W_v.T
    wp_d = nc.dram_tensor("wp", (YD, DIM), F32, kind="ExternalInput")       # w_proj[:, cols].T
    cos_d = nc.dram_tensor("costab", (YD, T), F16, kind="ExternalInput")
    sin_d = nc.dram_tensor("sintab", (YD, T), F16, kind="ExternalInput")
    mask_d = nc.dram_tensor("maskt", (HPC, 2, P, CHUNK), F16, kind="ExternalInput")
    qdec_d = nc.dram_tensor("qdec", (HPC, P, CHUNK), F32, kind="ExternalInput")
    kdec_d = nc.dram_tensor("kdec", (HPC, 2, P), F32, kind="ExternalInput")
    lamc_d = nc.dram_tensor("lamc", (HPC, P), F32, kind="ExternalInput")
    out_d = nc.dram_tensor("out", (T, DIM), F32, kind="ExternalOutput")

    with tile.TileContext(nc) as tc:
        with (
            tc.tile_pool(name="const", bufs=1) as constp,
            tc.tile_pool(name="dram", bufs=1, space="DRAM") as dram,
        ):
            # ---- constants (small) ----
            ident16 = constp.tile([P, P], F16)
            from concourse.masks import make_identity
            make_identity(nc, ident16)
            kdec_t = constp.tile([P, HPC, 2], F32)
            nc.sync.dma_start(kdec_t, kdec_d.ap().rearrange("h j p -> p h j"))
            lamc_t = constp.tile([P, HPC], F32)
            nc.sync.dma_start(lamc_t, lamc_d.ap().rearrange("h p -> p h"))

            qkTs = []
            for _qi in range(QK_DIMS // P):
                qkTs.append(dram.tile([P, T], F16, name=f"qkT{_qi}", tag=f"qkT{_qi}"))
            vnd = dram.tile([T, YD], F16)
            ytd = dram.tile([YD, T], F32)

            env = dict(locals())
            loop_cm = tc.For_i(0, loop_n, 1) if loop_n > 1 else contextlib.nullcontext()
            with loop_cm:
                _phases(nc, tc, env, phases)

    nc.compile()
    return nc


def _phases(nc, tc, env, which="123"):
    mult = mybir.AluOpType.mult
    add = mybir.AluOpType.add
    COPY = mybir.ActivationFunctionType.Copy
    xt_d = env["xt_d"]; wt_d = env["wt_d"]; wv_d = env["wv_d"]; wp_d = env["wp_d"]
    cos_d = env["cos_d"]; sin_d = env["sin_d"]
    mask_d = env["mask_d"]; qdec_d = env["qdec_d"]
    kdec_t = env["kdec_t"]; lamc_t = env["lamc_t"]; ident16 = env["ident16"]
    qkTs = env["qkTs"]; vnd = env["vnd"]; ytd = env["ytd"]; out_d = env["out_d"]

    if "1" in which:
        # ====== Phase 1: qkT = W_qk @ x_b.T  and  v_nat = x_b @ W_v.T ======
        with (
            tc.tile_pool(name="p1x", bufs=1) as p1x,
            tc.tile_pool(name="p1w", bufs=2) as p1w,
            tc.tile_pool(name="p1v", bufs=1) as p1v,
            tc.tile_pool(name="p1ps", bufs=8, space="PSUM") as p1ps,
        ):
            xt = p1x.tile([P, KC, T], F16, tag="xt")
            xt_src = xt_d.ap().rearrange("(kc p) t -> p kc t", p=P)
            for kc in range(KC):
                nc.sync.dma_start(xt[:, kc], xt_src[:, kc])

            # v first (head 0's attention needs it), natural layout
            wv_src = wv_d.ap().rearrange("(kc p) m -> p kc m", p=P)
            for nv in range(2):                     # vdims 0:512, 512:1024
                wvt = p1v.tile([P, KC, 512], F16, tag="wv")
                nc.sync.dma_start(wvt, wv_src[:, :, nv * 512:(nv + 1) * 512])
                for mt in range(T // P):            # 16 token tiles
                    ps = p1ps.tile([P, 512], F32, tag="p1")
                    for kc in range(KC):
                        nc.tensor.matmul(ps, xt[:, kc, mt * P:(mt + 1) * P],
                                         wvt[:, kc],
                                         start=(kc == 0), stop=(kc == KC - 1))
                    so = p1w.tile([P, 512], F16, tag="p1o")
                    nc.any.tensor_copy(out=so, in_=ps)
                    nc.scalar.dma_start(
                        vnd[mt * P:(mt + 1) * P, nv * 512:(nv + 1) * 512], so)

            # qk in T-layout, m-tiles interleaved by head so head h's q/k
            # finish early and its attention overlaps the rest of phase 1
            wt_src = wt_d.ap().rearrange("(kc p) m -> p kc m", p=P)
            m_order = [mm for h in range(HPC) for mm in (h, HPC + h)]
            for m in m_order:                       # 16
                wm = p1w.tile([P, KC, P], F16, tag="wm")
                nc.sync.dma_start(wm, wt_src[:, :, m * P:(m + 1) * P])
                for n in range(NT):                 # 4
                    ps = p1ps.tile([P, 512], F32, tag="p1")
                    for kc in range(KC):            # 16
                        nc.tensor.matmul(ps, wm[:, kc],
                                         xt[:, kc, n * 512:(n + 1) * 512],
                                         start=(kc == 0), stop=(kc == KC - 1))
                    so = p1w.tile([P, 512], F16, tag="p1o")
                    nc.any.tensor_copy(out=so, in_=ps)
                    nc.scalar.dma_start(
                        qkTs[m][:, n * 512:(n + 1) * 512], so)

    if "2" in which:
        # ================= Phase 2: attention per head =================
        with (
            tc.tile_pool(name="p2cst", bufs=1) as p2cst,
            tc.tile_pool(name="p2io", bufs=3) as p2io,
            tc.tile_pool(name="p2c", bufs=3) as p2c,
            tc.tile_pool(name="p2s", bufs=2) as p2s,
            tc.tile_pool(name="p2ps", bufs=2, space="PSUM") as p2ps,
            tc.tile_pool(name="p2ps1", bufs=1, space="PSUM") as p2ps1,
        ):
            mask_t = p2cst.tile([P, HPC, 2, CHUNK], F16)
            nc.sync.dma_start(mask_t, mask_d.ap().rearrange("h j p c -> p h j c"))
            qdec_t = p2cst.tile([P, HPC, CHUNK], F32)
            nc.sync.dma_start(qdec_t, qdec_d.ap().rearrange("h p c -> p h c"))

            for h in range(HPC):
                r = h * P
                qt = p2io.tile([P, T], F16, tag="qt")
                kt = p2io.tile([P, T], F16, tag="kt")
                cost = p2io.tile([P, T], F16, tag="cost")
                sint = p2io.tile([P, T], F16, tag="sint")
                vna = p2io.tile([P, KC, E], F16, tag="vna")
                nc.sync.dma_start(qt, qkTs[h][:])
                nc.sync.dma_start(kt, qkTs[HPC + h][:])
                nc.sync.dma_start(cost, cos_d.ap()[r:r + P])
                nc.sync.dma_start(sint, sin_d.ap()[r:r + P])
                nc.sync.dma_start(
                    vna, vnd[:, r:r + P].rearrange("(tt p) d -> p tt d", p=P))

                st0 = p2s.tile([P, E], F32R, tag="st0")
                st1 = p2s.tile([P, E], F32R, tag="st1")
                ylt = p2s.tile([P, T], F32R, tag="ylt")
                lam_col = lamc_t[:, h:h + 1]

                for i in range(NCH):
                    sl = slice(i * CHUNK, (i + 1) * CHUNK)
                    # LRPE for this chunk -> fp16
                    qlc = p2c.tile([P, CHUNK], F16, tag="qlc")
                    qls = p2c.tile([P, CHUNK], F16, tag="qls")
                    klc = p2c.tile([P, CHUNK], F16, tag="klc")
                    kls = p2c.tile([P, CHUNK], F16, tag="kls")
                    nc.vector.tensor_tensor(qlc, qt[:, sl], cost[:, sl], mult)
                    nc.vector.tensor_tensor(qls, qt[:, sl], sint[:, sl], mult)
                    nc.vector.tensor_tensor(klc, kt[:, sl], cost[:, sl], mult)
                    nc.vector.tensor_tensor(kls, kt[:, sl], sint[:, sl], mult)
                    # k natural layout via PE transpose (fp16) + k_decay scale
                    knat = p2c.tile([P, 2, 2 * E], F16, tag="knat")
                    for half in range(2):
                        hsl = slice(half * P, (half + 1) * P)
                        kd = kdec_t[:, h, half:half + 1]
                        pk0 = p2ps.tile([P, P], F16, tag="ptr")
                        nc.tensor.transpose(pk0, klc[:, hsl], ident16)
                        nc.scalar.activation(knat[:, half, 0:E], pk0, COPY,
                                             bias=0.0, scale=kd)
                        pk1 = p2ps.tile([P, P], F16, tag="ptr")
                        nc.tensor.transpose(pk1, kls[:, hsl], ident16)
                        nc.scalar.activation(knat[:, half, E:2 * E], pk1, COPY,
                                             bias=0.0, scale=kd)
                    # scoresT (two j-half tiles), mask multiply
                    smask = []
                    for jh in range(2):
                        jsl = slice(jh * P, (jh + 1) * P)
                        sps = p2ps.tile([P, CHUNK], F32, tag="sco")
                        nc.tensor.matmul(sps, klc[:, jsl], qlc,
                                         start=True, stop=False)
                        nc.tensor.matmul(sps, kls[:, jsl], qls,
                                         start=False, stop=True)
                        sm = p2c.tile([P, CHUNK], F16, tag=f"smask{jh}")
                        nc.vector.tensor_tensor(sm, sps, mask_t[:, h, jh], mult)
                        smask.append(sm)
                    # oT
                    ops = p2ps.tile([E, CHUNK], F32, tag="ops")
                    nc.tensor.matmul(ops, vna[:, 2 * i], smask[0],
                                     start=True, stop=False)
                    nc.tensor.matmul(ops, vna[:, 2 * i + 1], smask[1],
                                     start=False, stop=(i == 0))
                    if i > 0:
                        qsc = p2c.tile([P, CHUNK], F32R, tag="qsc")
                        qss = p2c.tile([P, CHUNK], F32R, tag="qss")
                        nc.vector.tensor_tensor(qsc, qlc, qdec_t[:, h], mult)
                        nc.vector.tensor_tensor(qss, qls, qdec_t[:, h], mult)
                        nc.tensor.matmul(ops, st0, qsc, start=False, stop=False)
                        nc.tensor.matmul(ops, st1, qss, start=False, stop=True)
                    nc.scalar.copy(ylt[:, sl], ops)
                    # state update (not needed after the last chunk)
                    if i < NCH - 1:
                        sadd0 = p2ps1.tile([E, E], F32, tag="sadd0")
                        sadd1 = p2ps1.tile([E, E], F32, tag="sadd1")
                        nc.tensor.matmul(sadd0, knat[:, 0, 0:E], vna[:, 2 * i],
                                         start=True, stop=False)
                        nc.tensor.matmul(sadd0, knat[:, 1, 0:E], vna[:, 2 * i + 1],
                                         start=False, stop=True)
                        nc.tensor.matmul(sadd1, knat[:, 0, E:2 * E], vna[:, 2 * i],
                                         start=True, stop=False)
                        nc.tensor.matmul(sadd1, knat[:, 1, E:2 * E], vna[:, 2 * i + 1],
                                         start=False, stop=True)
                        if i == 0:
                            nc.vector.tensor_copy(out=st0, in_=sadd0)
                            nc.vector.tensor_copy(out=st1, in_=sadd1)
                        else:
                            nc.vector.scalar_tensor_tensor(
                                out=st0, in0=st0.bitcast(F32), scalar=lam_col,
                                in1=sadd0, op0=mult, op1=add)
                            nc.vector.scalar_tensor_tensor(
                                out=st1, in0=st1.bitcast(F32), scalar=lam_col,
                                in1=sadd1, op0=mult, op1=add)
                nc.scalar.dma_start(ytd[r:r + P], ylt[:].bitcast(F32))

    if "3" in which:
        # ================= Phase 3: out = yT.T @ wpT =================
        with (
            tc.tile_pool(name="p3w", bufs=1) as p3w,
            tc.tile_pool(name="p3y", bufs=3) as p3y,
            tc.tile_pool(name="p3ps", bufs=8, space="PSUM") as p3ps,
        ):
            nkc3 = YD // P  # 8
            wp_src = wp_d.ap().rearrange("(kc p) n -> p kc n", p=P).bitcast(F32R)
            wpt = p3w.tile([P, nkc3, DIM], F32R, tag="wpt")
            for kc in range(nkc3):
                nc.sync.dma_start(wpt[:, kc], wp_src[:, kc])
            yt_src = ytd[:].rearrange("(kc p) t -> p kc t", p=P).bitcast(F32R)
            for m in range(T // P):                 # 16
                ym = p3y.tile([P, nkc3, P], F32R, tag="ym")
                nc.sync.dma_start(ym, yt_src[:, :, m * P:(m + 1) * P])
                for n in range(NT):                 # 4
                    ps = p3ps.tile([P, 512], F32, tag="p3")
                    for kc in range(nkc3):
                        nc.tensor.matmul(ps, ym[:, kc],
                                         wpt[:, kc, n * 512:(n + 1) * 512],
                                         start=(kc == 0), stop=(kc == nkc3 - 1))
                    so = p3y.tile([P, 512], F32, tag="p3o")
                    nc.any.tensor_copy(out=so, in_=ps)
                    nc.scalar.dma_start(
                        out_d.ap()[m * P:(m + 1) * P, n * 512:(n + 1) * 512], so)


def _get_nc():
    global _NC_CACHE
    if _NC_CACHE is None:
        _NC_CACHE = _build_nc()
    return _NC_CACHE


def _slopes(h):
    start = 2.0 ** (-(2.0 ** -(math.log2(h) - 3)))
    return np.array([start ** (i + 1) for i in range(h)], dtype=np.float64)


def _prepare_in_maps(x, w_qkv, w_proj, theta):
    slopes = _slopes(HEADS)
    t = np.arange(T, dtype=np.float64)
    idx = np.arange(CHUNK, dtype=np.float64)

    in_maps = []
    for core in range(8):
        b, g = divmod(core, 2)
        heads = np.arange(g * HPC, (g + 1) * HPC)

        xt = np.ascontiguousarray(x[b].T).astype(np.float16)

        qk_rows = np.concatenate([
            np.arange(g * YD, (g + 1) * YD),                 # q rows
            np.arange(DIM + g * YD, DIM + (g + 1) * YD),     # k rows
        ])
        wt = np.ascontiguousarray(w_qkv[qk_rows].T).astype(np.float16)
        v_rows = np.arange(2 * DIM + g * YD, 2 * DIM + (g + 1) * YD)
        wv = np.ascontiguousarray(w_qkv[v_rows].T).astype(np.float16)

        wp = np.ascontiguousarray(w_proj[:, g * YD:(g + 1) * YD].T, dtype=np.float32)

        th = theta.reshape(HEADS, E)[heads].astype(np.float64)  # (8, 128)
        ang = th[:, :, None] * t[None, None, :]                 # (8, 128, T)
        costab = np.cos(ang).astype(np.float16).reshape(YD, T)
        sintab = np.sin(ang).astype(np.float16).reshape(YD, T)

        s = slopes[heads]                                       # (8,)
        diff = idx[:, None] - idx[None, :]                      # (i, j)
        maskt = np.where(
            diff[None] >= 0, np.exp(-s[:, None, None] * diff[None]), 0.0
        )                                                       # (8, i, j) = diag_decay
        maskt = np.ascontiguousarray(
            maskt.transpose(0, 2, 1).reshape(HPC, 2, P, CHUNK)).astype(np.float16)
        qdec = np.exp(-s[:, None] * (idx + 1.0)[None]).astype(np.float32)  # (8, 256)
        qdec = np.broadcast_to(qdec[:, None, :], (HPC, P, CHUNK)).copy()
        kdec = np.exp(-s[:, None] * (CHUNK - 1.0 - idx)[None]).astype(np.float32)
        kdec = np.ascontiguousarray(kdec.reshape(HPC, 2, P))
        lamc = np.exp(-s * CHUNK).astype(np.float32)            # (8,)
        lamc = np.broadcast_to(lamc[:, None], (HPC, P)).copy()

        in_maps.append({
            "xt": xt, "wt": wt, "wv": wv, "wp": wp,
            "costab": costab, "sintab": sintab,
            "maskt": maskt, "qdec": qdec, "kdec": kdec, "lamc": lamc,
        })
    return in_maps


def kernel(x, w_qkv, w_proj, theta):
    x = np.asarray(x)
    w_qkv = np.asarray(w_qkv)
    w_proj = np.asarray(w_proj)
    theta = np.asarray(theta)

    nc = _get_nc()
    in_maps = _prepare_in_maps(x, w_qkv, w_proj, theta)
    res = bass_utils.run_bass_kernel_spmd(nc, in_maps, core_ids=list(range(8)))

    out = np.empty((B, T, DIM), dtype=np.float32)
    for b in range(B):
        out[b] = res.results[2 * b]["out"] + res.results[2 * b + 1]["out"]
    return out
